# revision 11
# baseline (speedup 1.0000x reference)
"""Trainium2 Bass kernel for nn_Contracter (per-channel bilinear CG contraction).

out[z,u,k] = sum_ij x1[z,u,i] * x2[z,u,j] * ww3j[u,k,i,j]
with ww3j[u,k,i,j] = sum_p weights[u,p] * w3j[p,k,i,j]

Z=50000 edges, u=64 channels, i/j/k=9 (0e+1o+2e irreps).

Strategy (8 cores, data-parallel over z):
 - Host folds weights into w3j (tiny) and permutes x1/x2 so that SBUF
   partitions hold (edge-half h, channel u) = 128 lanes, and the free dim
   holds (chunk, irrep-component i, edge-within-chunk). Per-channel CG
   coefficients then become per-partition scalars.
 - v1 (KVER=1): VectorE does everything: 71 tensor_tensor products + 83
   fused scalar_tensor_tensor MACs per chunk.
 - v2 (KVER=2, default): VectorE forms the 71 raw products; ScalarE applies
   the per-channel coefficient (83 scaled copies, float32r out); TensorE
   accumulates the scaled planes into per-k PSUM banks via an
   identity-stationary float32r matmul; ScalarE drains each bank.
 - Host inverse-permutes the output.
"""

import os
import numpy as np

import concourse.bacc as bacc
import concourse.mybir as mybir
from concourse.tile import TileContext
from concourse.bass_utils import run_bass_kernel_spmd

MUL = 64
BASE = 9
Z = 50000
NCORES = 8
ZPC = Z // NCORES          # 6250 edges per core
NCHUNK = 7                 # chunks per half
ZC = 448                   # edges per chunk per half
ZH = NCHUNK * ZC           # 3136 padded half
ZPAD = 2 * ZH              # 6272 padded edges per core

F32 = mybir.dt.float32
F32R = mybir.dt.float32r


def _emission_order(w3j):
    """Sparsity of the CG tensor: (i,j)-pair groups with their output k's."""
    nz = (np.abs(np.asarray(w3j)) > 0).any(axis=0)  # (k,i,j)
    by_pair = {}
    for k in range(BASE):
        for i in range(BASE):
            for j in range(BASE):
                if nz[k, i, j]:
                    by_pair.setdefault((i, j), []).append(k)
    return [(i, j, sorted(by_pair[(i, j)])) for (i, j) in sorted(by_pair)]


def _new_nc():
    return bacc.Bacc("TRN2", target_bir_lowering=False, debug=False)


def _build_bass(emit):
    ntrip = sum(len(ks) for (_, _, ks) in emit)
    nc = _new_nc()
    xind = nc.dram_tensor("xin", [128, NCHUNK, 2, BASE, ZC], F32, kind="ExternalInput")
    cfd = nc.dram_tensor("coef", [128, ntrip], F32, kind="ExternalInput")
    outd = nc.dram_tensor("outp", [128, NCHUNK, BASE, ZC], F32, kind="ExternalOutput")

    with TileContext(nc) as tc:
        with (
            tc.tile_pool(name="const", bufs=1) as cpool,
            tc.tile_pool(name="io", bufs=2) as iopool,
            tc.tile_pool(name="prod", bufs=4) as ppool,
        ):
            coeft = cpool.tile([128, ntrip], F32)
            nc.sync.dma_start(out=coeft[:], in_=cfd[:])

            for c in range(NCHUNK):
                xt = iopool.tile([128, 2, BASE, ZC], F32, tag="x")
                acct = iopool.tile([128, BASE, ZC], F32, tag="acc")
                nc.sync.dma_start(out=xt[:], in_=xind[:, c])
                x1t = xt[:, 0]
                x2t = xt[:, 1]
                nc.vector.memset(acct[:], 0.0)

                t = 0
                for (i, j, ks) in emit:
                    pt = ppool.tile([128, ZC], F32, tag="p")
                    nc.vector.tensor_mul(pt[:], x1t[:, i], x2t[:, j])
                    for k in ks:
                        sc = coeft[:, t : t + 1]
                        nc.vector.scalar_tensor_tensor(
                            out=acct[:, k],
                            in0=pt[:],
                            scalar=sc,
                            in1=acct[:, k],
                            op0=mybir.AluOpType.mult,
                            op1=mybir.AluOpType.add,
                        )
                        t += 1
                nc.sync.dma_start(out=outd[:, c], in_=acct[:])
    nc.compile()
    return nc


def _build_bass_v2(emit):
    """v2: VectorE forms raw products (71 TT); ScalarE applies the per-channel
    CG coefficient (83 scaled copies, fp32r out); TensorE accumulates the
    scaled planes into per-k PSUM banks via identity-stationary float32r
    matmuls; ScalarE drains each bank. k-major emission keeps PSUM usage low.
    """
    ntrip = sum(len(ks) for (_, _, ks) in emit)

    by_k = {k: [] for k in range(BASE)}
    for (i, j, ks) in emit:
        for k in ks:
            by_k[k].append((i, j))
    tidx = {}
    t = 0
    for k in range(BASE):
        for (i, j) in by_k[k]:
            tidx[(k, i, j)] = t
            t += 1

    nc = _new_nc()
    xind = nc.dram_tensor("xin", [128, NCHUNK, 2, BASE, ZC], F32, kind="ExternalInput")
    cfd = nc.dram_tensor("coef", [128, ntrip], F32, kind="ExternalInput")
    idd = nc.dram_tensor("ident", [128, 128], F32R, kind="ExternalInput")
    outd = nc.dram_tensor("outp", [128, NCHUNK, BASE, ZC], F32, kind="ExternalOutput")

    with TileContext(nc) as tc:
        with (
            tc.tile_pool(name="const", bufs=1) as cpool,
            tc.tile_pool(name="io", bufs=2) as iopool,
            tc.tile_pool(name="mprod", bufs=14) as mpool,
            tc.tile_pool(name="q", bufs=14) as qpool,
            tc.tile_pool(name="ps", bufs=3, space="PSUM") as pspool,
        ):
            coeft = cpool.tile([128, ntrip], F32)
            nc.sync.dma_start(out=coeft[:], in_=cfd[:])
            identt = cpool.tile([128, 128], F32R)
            nc.sync.dma_start(out=identt[:], in_=idd[:])

            for c in range(NCHUNK):
                xt = iopool.tile([128, 2, BASE, ZC], F32, tag="x")
                outt = iopool.tile([128, BASE, ZC], F32, tag="out")
                nc.sync.dma_start(out=xt[:], in_=xind[:, c])
                x1t = xt[:, 0]
                x2t = xt[:, 1]

                mprod = {}
                for k in range(BASE):
                    pairs_k = by_k[k]
                    ps = pspool.tile([128, ZC], F32, tag="acc")
                    for idx, (i, j) in enumerate(pairs_k):
                        sc = coeft[:, tidx[(k, i, j)] : tidx[(k, i, j)] + 1]
                        if (i, j) not in mprod:
                            mt = mpool.tile([128, ZC], F32, tag="m")
                            nc.vector.tensor_mul(mt[:], x1t[:, i], x2t[:, j])
                            mprod[(i, j)] = mt
                        qt = qpool.tile([128, ZC], F32R, tag="q")
                        nc.scalar.mul(qt[:], mprod[(i, j)][:], sc)
                        nc.tensor.matmul(
                            out=ps[:],
                            lhsT=identt[:],
                            rhs=qt[:],
                            start=(idx == 0),
                            stop=(idx == len(pairs_k) - 1),
                        )
                    nc.scalar.copy(outt[:, k], ps[:])
                nc.sync.dma_start(out=outd[:, c], in_=outt[:])
    nc.compile()
    return nc




BF16 = mybir.dt.bfloat16


def _build_bass_v3(emit):
    """v3: like v2 but bf16 inputs/products for DVE 2x mode. Single-k pairs
    fuse product+coefficient into one scalar_tensor_tensor on VectorE;
    multi-k pairs take a raw TT product + per-k ScalarE scaled copies.
    TensorE accumulates bf16 planes into fp32 PSUM via identity matmuls."""
    ntrip = sum(len(ks) for (_, _, ks) in emit)

    pair_ks = {(i, j): ks for (i, j, ks) in emit}
    by_k = {k: [] for k in range(BASE)}
    for (i, j, ks) in emit:
        for k in ks:
            by_k[k].append((i, j))
    tidx = {}
    t = 0
    for k in range(BASE):
        for (i, j) in by_k[k]:
            tidx[(k, i, j)] = t
            t += 1

    nc = _new_nc()
    xind = nc.dram_tensor("xin", [128, NCHUNK, 2, BASE, ZC], BF16, kind="ExternalInput")
    cfd = nc.dram_tensor("coef", [128, ntrip], F32, kind="ExternalInput")
    idd = nc.dram_tensor("ident", [128, 128], BF16, kind="ExternalInput")
    outd = nc.dram_tensor("outp", [128, NCHUNK, BASE, ZC], F32, kind="ExternalOutput")

    with TileContext(nc) as tc:
        with (
            tc.tile_pool(name="const", bufs=1) as cpool,
            tc.tile_pool(name="io", bufs=2) as iopool,
            tc.tile_pool(name="mprod", bufs=14) as mpool,
            tc.tile_pool(name="q", bufs=14) as qpool,
            tc.tile_pool(name="ps", bufs=3, space="PSUM") as pspool,
        ):
            coeft = cpool.tile([128, ntrip], F32)
            nc.sync.dma_start(out=coeft[:], in_=cfd[:])
            identt = cpool.tile([128, 128], BF16)
            nc.sync.dma_start(out=identt[:], in_=idd[:])

            for c in range(NCHUNK):
                xt = iopool.tile([128, 2, BASE, ZC], BF16, tag="x")
                outt = iopool.tile([128, BASE, ZC], F32, tag="out")
                nc.sync.dma_start(out=xt[:], in_=xind[:, c])
                x1t = xt[:, 0]
                x2t = xt[:, 1]

                mprod = {}
                for k in range(BASE):
                    pairs_k = by_k[k]
                    ps = pspool.tile([128, ZC], F32, tag="acc")
                    for idx, (i, j) in enumerate(pairs_k):
                        sc = coeft[:, tidx[(k, i, j)] : tidx[(k, i, j)] + 1]
                        if len(pair_ks[(i, j)]) == 1:
                            qt = qpool.tile([128, ZC], BF16, tag="q")
                            nc.vector.scalar_tensor_tensor(
                                out=qt[:],
                                in0=x1t[:, i],
                                scalar=sc,
                                in1=x2t[:, j],
                                op0=mybir.AluOpType.mult,
                                op1=mybir.AluOpType.mult,
                            )
                        else:
                            if (i, j) not in mprod:
                                mt = mpool.tile([128, ZC], BF16, tag="m")
                                nc.vector.tensor_mul(mt[:], x1t[:, i], x2t[:, j])
                                mprod[(i, j)] = mt
                            qt = qpool.tile([128, ZC], BF16, tag="q")
                            nc.scalar.mul(qt[:], mprod[(i, j)][:], sc)
                        nc.tensor.matmul(
                            out=ps[:],
                            lhsT=identt[:],
                            rhs=qt[:],
                            start=(idx == 0),
                            stop=(idx == len(pairs_k) - 1),
                        )
                    nc.scalar.copy(outt[:, k], ps[:])
                nc.sync.dma_start(out=outd[:, c], in_=outt[:])
    nc.compile()
    return nc


def _coef_order_v2(emit):
    by_k = {k: [] for k in range(BASE)}
    for (i, j, ks) in emit:
        for k in ks:
            by_k[k].append((i, j))
    return [(k, i, j) for k in range(BASE) for (i, j) in by_k[k]]


def _coef_order_v1(emit):
    return [(k, i, j) for (i, j, ks) in emit for k in ks]


_CACHED = {}


def _permute_core(x_core_pad):
    """(ZPAD, 64, 9) -> (128, NCHUNK, 9, ZC) with partition p = h*64+u."""
    v = x_core_pad.reshape(2, NCHUNK, ZC, MUL, BASE)
    v = v.transpose(0, 3, 1, 4, 2)  # (h, u, chunk, i, zl)
    return np.ascontiguousarray(v.reshape(128, NCHUNK, BASE, ZC), dtype=np.float32)


def _unpermute_core(o_dev):
    """(128, NCHUNK, 9, ZC) -> (ZPAD, 64, 9)."""
    v = o_dev.reshape(2, MUL, NCHUNK, BASE, ZC)
    v = v.transpose(0, 2, 4, 1, 3)  # (h, chunk, zl, u, k)
    return v.reshape(ZPAD, MUL, BASE)


def kernel(x1, x2, weights, w3j):
    x1 = np.asarray(x1, dtype=np.float32)
    x2 = np.asarray(x2, dtype=np.float32)
    weights = np.asarray(weights, dtype=np.float32)
    w3j = np.asarray(w3j, dtype=np.float32)

    ver = os.environ.get("KVER", "2")

    # fold path weights into the CG tensor (tiny host einsum)
    ww3j = np.einsum("up,pkij->ukij", weights, w3j).astype(np.float32)

    emit = _emission_order(w3j)
    order = _coef_order_v1(emit) if ver == "1" else _coef_order_v2(emit)
    coef_u = np.stack([ww3j[:, k, i, j] for (k, i, j) in order], axis=1)  # (64,T)
    coef = np.ascontiguousarray(
        np.concatenate([coef_u, coef_u], axis=0), dtype=np.float32
    )  # (128, T)
    import ml_dtypes
    idt = np.float32 if ver == "2" else ml_dtypes.bfloat16
    ident = np.ascontiguousarray(np.eye(128, dtype=idt))

    x1r = x1.reshape(Z, MUL, BASE)
    x2r = x2.reshape(Z, MUL, BASE)

    in_maps = []
    for c in range(NCORES):
        sl = slice(c * ZPC, (c + 1) * ZPC)
        x1c = np.zeros((ZPAD, MUL, BASE), np.float32)
        x2c = np.zeros((ZPAD, MUL, BASE), np.float32)
        x1c[:ZPC] = x1r[sl]
        x2c[:ZPC] = x2r[sl]
        xin = np.ascontiguousarray(
            np.stack([_permute_core(x1c), _permute_core(x2c)], axis=2)
        )  # (128, NCHUNK, 2, BASE, ZC)
        if ver == "3":
            xin = xin.astype(ml_dtypes.bfloat16)
        if ver == "1":
            in_maps.append({"xin": xin, "coef": coef})
        else:
            in_maps.append({"xin": xin, "coef": coef, "ident": ident})

    key = (ver,) + tuple((i, j, tuple(ks)) for (i, j, ks) in emit)
    if _CACHED.get("key") != key:
        build = {"1": _build_bass, "2": _build_bass_v2, "3": _build_bass_v3}[ver]
        _CACHED["nc"] = build(emit)
        _CACHED["key"] = key
    nc = _CACHED["nc"]

    trace = os.environ.get("BASS_TRACE", "0") == "1"
    res = run_bass_kernel_spmd(
        nc, in_maps, core_ids=list(range(NCORES)), trace=trace
    )
    _CACHED["last_results"] = res
    _CACHED["nc_inmaps"] = (nc, in_maps)

    out = np.empty((Z, MUL, BASE), np.float32)
    for c in range(NCORES):
        o = _unpermute_core(res.results[c]["outp"])
        out[c * ZPC : (c + 1) * ZPC] = o[:ZPC]
    return out


# revision 14
# speedup vs baseline: 2.1295x; 2.1295x over previous
"""Trainium2 Bass kernel for nn_Contracter (per-channel bilinear CG contraction).

out[z,u,k] = sum_ij x1[z,u,i] * x2[z,u,j] * ww3j[u,k,i,j]
with ww3j[u,k,i,j] = sum_p weights[u,p] * w3j[p,k,i,j]

Z=50000 edges, u=64 channels, i/j/k=9 (0e+1o+2e irreps).

Strategy (8 cores, data-parallel over z):
 - Host folds weights into w3j (tiny) and permutes x1/x2 so that SBUF
   partitions hold (edge-half h, channel u) = 128 lanes, and the free dim
   holds (chunk, irrep-component i, edge-within-chunk). Per-channel CG
   coefficients then become per-partition scalars.
 - v1 (KVER=1): VectorE does everything: 71 tensor_tensor products + 83
   fused scalar_tensor_tensor MACs per chunk.
 - v2 (KVER=2, default): VectorE forms the 71 raw products; ScalarE applies
   the per-channel coefficient (83 scaled copies, float32r out); TensorE
   accumulates the scaled planes into per-k PSUM banks via an
   identity-stationary float32r matmul; ScalarE drains each bank.
 - Host inverse-permutes the output.
"""

import os
import numpy as np

import concourse.bacc as bacc
import concourse.mybir as mybir
from concourse.tile import TileContext
from concourse.bass_utils import run_bass_kernel_spmd

MUL = 64
BASE = 9
Z = 50000
NCORES = 8
ZPC = Z // NCORES          # 6250 edges per core
NCHUNK = 7                 # chunks per half
ZC = 448                   # edges per chunk per half
ZH = NCHUNK * ZC           # 3136 padded half
ZPAD = 2 * ZH              # 6272 padded edges per core

F32 = mybir.dt.float32
F32R = mybir.dt.float32r


def _emission_order(w3j):
    """Sparsity of the CG tensor: (i,j)-pair groups with their output k's."""
    nz = (np.abs(np.asarray(w3j)) > 0).any(axis=0)  # (k,i,j)
    by_pair = {}
    for k in range(BASE):
        for i in range(BASE):
            for j in range(BASE):
                if nz[k, i, j]:
                    by_pair.setdefault((i, j), []).append(k)
    return [(i, j, sorted(by_pair[(i, j)])) for (i, j) in sorted(by_pair)]


def _new_nc():
    return bacc.Bacc("TRN2", target_bir_lowering=False, debug=False)


def _build_bass(emit):
    ntrip = sum(len(ks) for (_, _, ks) in emit)
    nc = _new_nc()
    xind = nc.dram_tensor("xin", [128, NCHUNK, 2, BASE, ZC], F32, kind="ExternalInput")
    cfd = nc.dram_tensor("coef", [128, ntrip], F32, kind="ExternalInput")
    outd = nc.dram_tensor("outp", [128, NCHUNK, BASE, ZC], F32, kind="ExternalOutput")

    with TileContext(nc) as tc:
        with (
            tc.tile_pool(name="const", bufs=1) as cpool,
            tc.tile_pool(name="io", bufs=2) as iopool,
            tc.tile_pool(name="prod", bufs=4) as ppool,
        ):
            coeft = cpool.tile([128, ntrip], F32)
            nc.sync.dma_start(out=coeft[:], in_=cfd[:])

            for c in range(NCHUNK):
                xt = iopool.tile([128, 2, BASE, ZC], F32, tag="x")
                acct = iopool.tile([128, BASE, ZC], F32, tag="acc")
                nc.sync.dma_start(out=xt[:], in_=xind[:, c])
                x1t = xt[:, 0]
                x2t = xt[:, 1]
                nc.vector.memset(acct[:], 0.0)

                t = 0
                for (i, j, ks) in emit:
                    pt = ppool.tile([128, ZC], F32, tag="p")
                    nc.vector.tensor_mul(pt[:], x1t[:, i], x2t[:, j])
                    for k in ks:
                        sc = coeft[:, t : t + 1]
                        nc.vector.scalar_tensor_tensor(
                            out=acct[:, k],
                            in0=pt[:],
                            scalar=sc,
                            in1=acct[:, k],
                            op0=mybir.AluOpType.mult,
                            op1=mybir.AluOpType.add,
                        )
                        t += 1
                nc.sync.dma_start(out=outd[:, c], in_=acct[:])
    nc.compile()
    return nc


def _build_bass_v2(emit):
    """v2: VectorE forms raw products (71 TT); ScalarE applies the per-channel
    CG coefficient (83 scaled copies, fp32r out); TensorE accumulates the
    scaled planes into per-k PSUM banks via identity-stationary float32r
    matmuls; ScalarE drains each bank. k-major emission keeps PSUM usage low.
    """
    ntrip = sum(len(ks) for (_, _, ks) in emit)

    by_k = {k: [] for k in range(BASE)}
    for (i, j, ks) in emit:
        for k in ks:
            by_k[k].append((i, j))
    tidx = {}
    t = 0
    for k in range(BASE):
        for (i, j) in by_k[k]:
            tidx[(k, i, j)] = t
            t += 1

    nc = _new_nc()
    xind = nc.dram_tensor("xin", [128, NCHUNK, 2, BASE, ZC], F32, kind="ExternalInput")
    cfd = nc.dram_tensor("coef", [128, ntrip], F32, kind="ExternalInput")
    idd = nc.dram_tensor("ident", [128, 128], F32R, kind="ExternalInput")
    outd = nc.dram_tensor("outp", [128, NCHUNK, BASE, ZC], F32, kind="ExternalOutput")

    with TileContext(nc) as tc:
        with (
            tc.tile_pool(name="const", bufs=1) as cpool,
            tc.tile_pool(name="io", bufs=2) as iopool,
            tc.tile_pool(name="mprod", bufs=14) as mpool,
            tc.tile_pool(name="q", bufs=14) as qpool,
            tc.tile_pool(name="ps", bufs=3, space="PSUM") as pspool,
        ):
            coeft = cpool.tile([128, ntrip], F32)
            nc.sync.dma_start(out=coeft[:], in_=cfd[:])
            identt = cpool.tile([128, 128], F32R)
            nc.sync.dma_start(out=identt[:], in_=idd[:])

            for c in range(NCHUNK):
                xt = iopool.tile([128, 2, BASE, ZC], F32, tag="x")
                outt = iopool.tile([128, BASE, ZC], F32, tag="out")
                nc.sync.dma_start(out=xt[:], in_=xind[:, c])
                x1t = xt[:, 0]
                x2t = xt[:, 1]

                mprod = {}
                for k in range(BASE):
                    pairs_k = by_k[k]
                    ps = pspool.tile([128, ZC], F32, tag="acc")
                    for idx, (i, j) in enumerate(pairs_k):
                        sc = coeft[:, tidx[(k, i, j)] : tidx[(k, i, j)] + 1]
                        if (i, j) not in mprod:
                            mt = mpool.tile([128, ZC], F32, tag="m")
                            nc.vector.tensor_mul(mt[:], x1t[:, i], x2t[:, j])
                            mprod[(i, j)] = mt
                        qt = qpool.tile([128, ZC], F32R, tag="q")
                        nc.scalar.mul(qt[:], mprod[(i, j)][:], sc)
                        nc.tensor.matmul(
                            out=ps[:],
                            lhsT=identt[:],
                            rhs=qt[:],
                            start=(idx == 0),
                            stop=(idx == len(pairs_k) - 1),
                        )
                    nc.scalar.copy(outt[:, k], ps[:])
                nc.sync.dma_start(out=outd[:, c], in_=outt[:])
    nc.compile()
    return nc




BF16 = mybir.dt.bfloat16


def _build_bass_v3(emit):
    """v3: like v2 but bf16 inputs/products for DVE 2x mode. Single-k pairs
    fuse product+coefficient into one scalar_tensor_tensor on VectorE;
    multi-k pairs take a raw TT product + per-k ScalarE scaled copies.
    TensorE accumulates bf16 planes into fp32 PSUM via identity matmuls."""
    ntrip = sum(len(ks) for (_, _, ks) in emit)

    pair_ks = {(i, j): ks for (i, j, ks) in emit}
    by_k = {k: [] for k in range(BASE)}
    for (i, j, ks) in emit:
        for k in ks:
            by_k[k].append((i, j))
    tidx = {}
    t = 0
    for k in range(BASE):
        for (i, j) in by_k[k]:
            tidx[(k, i, j)] = t
            t += 1

    nc = _new_nc()
    xind = nc.dram_tensor("xin", [128, NCHUNK, 2, BASE, ZC], BF16, kind="ExternalInput")
    cfd = nc.dram_tensor("coef", [128, ntrip], F32, kind="ExternalInput")
    idd = nc.dram_tensor("ident", [128, 128], BF16, kind="ExternalInput")
    outd = nc.dram_tensor("outp", [128, NCHUNK, BASE, ZC], F32, kind="ExternalOutput")

    with TileContext(nc) as tc:
        with (
            tc.tile_pool(name="const", bufs=1) as cpool,
            tc.tile_pool(name="io", bufs=2) as iopool,
            tc.tile_pool(name="mprod", bufs=14) as mpool,
            tc.tile_pool(name="q", bufs=14) as qpool,
            tc.tile_pool(name="ps", bufs=3, space="PSUM") as pspool,
        ):
            coeft = cpool.tile([128, ntrip], F32)
            nc.sync.dma_start(out=coeft[:], in_=cfd[:])
            identt = cpool.tile([128, 128], BF16)
            nc.sync.dma_start(out=identt[:], in_=idd[:])

            for c in range(NCHUNK):
                xt = iopool.tile([128, 2, BASE, ZC], BF16, tag="x")
                outt = iopool.tile([128, BASE, ZC], F32, tag="out")
                nc.sync.dma_start(out=xt[:], in_=xind[:, c])
                x1t = xt[:, 0]
                x2t = xt[:, 1]

                mprod = {}
                for k in range(BASE):
                    pairs_k = by_k[k]
                    ps = pspool.tile([128, ZC], F32, tag="acc")
                    for idx, (i, j) in enumerate(pairs_k):
                        sc = coeft[:, tidx[(k, i, j)] : tidx[(k, i, j)] + 1]
                        if (i, j) not in mprod:
                            mt = mpool.tile([128, ZC], BF16, tag="m")
                            nc.vector.tensor_mul(mt[:], x1t[:, i], x2t[:, j])
                            mprod[(i, j)] = mt
                        qt = qpool.tile([128, ZC], BF16, tag="q")
                        nc.scalar.mul(qt[:], mprod[(i, j)][:], sc)
                        nc.tensor.matmul(
                            out=ps[:],
                            lhsT=identt[:],
                            rhs=qt[:],
                            start=(idx == 0),
                            stop=(idx == len(pairs_k) - 1),
                        )
                    nc.scalar.copy(outt[:, k], ps[:])
                nc.sync.dma_start(out=outd[:, c], in_=outt[:])
    nc.compile()
    return nc




def _build_bass_v4(emit):
    """v4: per-channel coefficients ride the PE stationary as diagonal
    matrices (bf16). VectorE: 71 raw bf16 products. TensorE: 83
    diag-stationary matmuls accumulating into per-k PSUM banks. ScalarE:
    9 PSUM drains. No per-triple scaling op on any engine."""
    ntrip = sum(len(ks) for (_, _, ks) in emit)

    by_k = {k: [] for k in range(BASE)}
    for (i, j, ks) in emit:
        for k in ks:
            by_k[k].append((i, j))
    tidx = {}
    t = 0
    for k in range(BASE):
        for (i, j) in by_k[k]:
            tidx[(k, i, j)] = t
            t += 1

    nc = _new_nc()
    xind = nc.dram_tensor("xin", [128, NCHUNK, 2, BASE, ZC], BF16, kind="ExternalInput")
    dgd = nc.dram_tensor("diags", [128, ntrip, 128], BF16, kind="ExternalInput")
    outd = nc.dram_tensor("outp", [128, NCHUNK, BASE, ZC], F32, kind="ExternalOutput")

    with TileContext(nc) as tc:
        with (
            tc.tile_pool(name="const", bufs=1) as cpool,
            tc.tile_pool(name="io", bufs=2) as iopool,
            tc.tile_pool(name="mprod", bufs=14) as mpool,
            tc.tile_pool(name="ps", bufs=3, space="PSUM") as pspool,
        ):
            diagt = cpool.tile([128, ntrip, 128], BF16)
            nc.sync.dma_start(out=diagt[:], in_=dgd[:])

            for c in range(NCHUNK):
                xt = iopool.tile([128, 2, BASE, ZC], BF16, tag="x")
                outt = iopool.tile([128, BASE, ZC], F32, tag="out")
                nc.sync.dma_start(out=xt[:], in_=xind[:, c])
                x1t = xt[:, 0]
                x2t = xt[:, 1]

                mprod = {}
                for k in range(BASE):
                    pairs_k = by_k[k]
                    ps = pspool.tile([128, ZC], F32, tag="acc")
                    for idx, (i, j) in enumerate(pairs_k):
                        if (i, j) not in mprod:
                            mt = mpool.tile([128, ZC], BF16, tag="m")
                            nc.vector.tensor_mul(mt[:], x1t[:, i], x2t[:, j])
                            mprod[(i, j)] = mt
                        nc.tensor.matmul(
                            out=ps[:],
                            lhsT=diagt[:, tidx[(k, i, j)]],
                            rhs=mprod[(i, j)][:],
                            start=(idx == 0),
                            stop=(idx == len(pairs_k) - 1),
                        )
                    nc.scalar.copy(outt[:, k], ps[:])
                nc.sync.dma_start(out=outd[:, c], in_=outt[:])
    nc.compile()
    return nc




def _build_bass_v5(emit):
    """v5: precision-safe diag variant. VectorE: 71 fp32 products written as
    float32r; TensorE: 83 float32r diag-stationary matmuls (1 cyc/col at
    N>=256); ScalarE: 9 PSUM drains. Error stays at fp32r rounding level."""
    ntrip = sum(len(ks) for (_, _, ks) in emit)

    by_k = {k: [] for k in range(BASE)}
    for (i, j, ks) in emit:
        for k in ks:
            by_k[k].append((i, j))
    tidx = {}
    t = 0
    for k in range(BASE):
        for (i, j) in by_k[k]:
            tidx[(k, i, j)] = t
            t += 1

    nc = _new_nc()
    xind = nc.dram_tensor("xin", [128, NCHUNK, 2, BASE, ZC], F32, kind="ExternalInput")
    dgd = nc.dram_tensor("diags", [128, ntrip, 128], F32R, kind="ExternalInput")
    outd = nc.dram_tensor("outp", [128, NCHUNK, BASE, ZC], F32, kind="ExternalOutput")

    with TileContext(nc) as tc:
        with (
            tc.tile_pool(name="const", bufs=1) as cpool,
            tc.tile_pool(name="io", bufs=2) as iopool,
            tc.tile_pool(name="mprod", bufs=14) as mpool,
            tc.tile_pool(name="ps", bufs=3, space="PSUM") as pspool,
        ):
            diagt = cpool.tile([128, ntrip, 128], F32R)
            nc.sync.dma_start(out=diagt[:], in_=dgd[:])

            for c in range(NCHUNK):
                xt = iopool.tile([128, 2, BASE, ZC], F32, tag="x")
                outt = iopool.tile([128, BASE, ZC], F32, tag="out")
                nc.sync.dma_start(out=xt[:], in_=xind[:, c])
                x1t = xt[:, 0]
                x2t = xt[:, 1]

                mprod = {}
                for k in range(BASE):
                    pairs_k = by_k[k]
                    ps = pspool.tile([128, ZC], F32, tag="acc")
                    for idx, (i, j) in enumerate(pairs_k):
                        if (i, j) not in mprod:
                            mt = mpool.tile([128, ZC], F32R, tag="m")
                            nc.vector.tensor_mul(mt[:], x1t[:, i], x2t[:, j])
                            mprod[(i, j)] = mt
                        nc.tensor.matmul(
                            out=ps[:],
                            lhsT=diagt[:, tidx[(k, i, j)]],
                            rhs=mprod[(i, j)][:],
                            start=(idx == 0),
                            stop=(idx == len(pairs_k) - 1),
                        )
                    nc.scalar.copy(outt[:, k], ps[:])
                nc.sync.dma_start(out=outd[:, c], in_=outt[:])
    nc.compile()
    return nc


def _coef_order_v2(emit):
    by_k = {k: [] for k in range(BASE)}
    for (i, j, ks) in emit:
        for k in ks:
            by_k[k].append((i, j))
    return [(k, i, j) for k in range(BASE) for (i, j) in by_k[k]]


def _coef_order_v1(emit):
    return [(k, i, j) for (i, j, ks) in emit for k in ks]


_CACHED = {}


def _permute_core(x_core_pad):
    """(ZPAD, 64, 9) -> (128, NCHUNK, 9, ZC) with partition p = h*64+u."""
    v = x_core_pad.reshape(2, NCHUNK, ZC, MUL, BASE)
    v = v.transpose(0, 3, 1, 4, 2)  # (h, u, chunk, i, zl)
    return np.ascontiguousarray(v.reshape(128, NCHUNK, BASE, ZC), dtype=np.float32)


def _unpermute_core(o_dev):
    """(128, NCHUNK, 9, ZC) -> (ZPAD, 64, 9)."""
    v = o_dev.reshape(2, MUL, NCHUNK, BASE, ZC)
    v = v.transpose(0, 2, 4, 1, 3)  # (h, chunk, zl, u, k)
    return v.reshape(ZPAD, MUL, BASE)


def kernel(x1, x2, weights, w3j):
    x1 = np.asarray(x1, dtype=np.float32)
    x2 = np.asarray(x2, dtype=np.float32)
    weights = np.asarray(weights, dtype=np.float32)
    w3j = np.asarray(w3j, dtype=np.float32)

    ver = os.environ.get("KVER", "2")

    # fold path weights into the CG tensor (tiny host einsum)
    ww3j = np.einsum("up,pkij->ukij", weights, w3j).astype(np.float32)

    emit = _emission_order(w3j)
    order = _coef_order_v1(emit) if ver == "1" else _coef_order_v2(emit)
    coef_u = np.stack([ww3j[:, k, i, j] for (k, i, j) in order], axis=1)  # (64,T)
    coef = np.ascontiguousarray(
        np.concatenate([coef_u, coef_u], axis=0), dtype=np.float32
    )  # (128, T)
    import ml_dtypes
    idt = np.float32 if ver == "2" else ml_dtypes.bfloat16
    ident = np.ascontiguousarray(np.eye(128, dtype=idt))
    if ver in ("4", "5"):
        T = coef.shape[1]
        diags = np.zeros((128, T, 128), np.float32)
        diags[np.arange(128)[:, None], np.arange(T)[None, :], np.arange(128)[:, None]] = coef
        if ver == "4":
            diags = np.ascontiguousarray(diags.astype(ml_dtypes.bfloat16))
        else:
            diags = np.ascontiguousarray(diags)

    x1r = x1.reshape(Z, MUL, BASE)
    x2r = x2.reshape(Z, MUL, BASE)

    in_maps = []
    for c in range(NCORES):
        sl = slice(c * ZPC, (c + 1) * ZPC)
        x1c = np.zeros((ZPAD, MUL, BASE), np.float32)
        x2c = np.zeros((ZPAD, MUL, BASE), np.float32)
        x1c[:ZPC] = x1r[sl]
        x2c[:ZPC] = x2r[sl]
        xin = np.ascontiguousarray(
            np.stack([_permute_core(x1c), _permute_core(x2c)], axis=2)
        )  # (128, NCHUNK, 2, BASE, ZC)
        if ver in ("3", "4"):
            xin = xin.astype(ml_dtypes.bfloat16)
        if ver == "1":
            in_maps.append({"xin": xin, "coef": coef})
        elif ver in ("4", "5"):
            in_maps.append({"xin": xin, "diags": diags})
        else:
            in_maps.append({"xin": xin, "coef": coef, "ident": ident})

    key = (ver,) + tuple((i, j, tuple(ks)) for (i, j, ks) in emit)
    if _CACHED.get("key") != key:
        build = {"1": _build_bass, "2": _build_bass_v2, "3": _build_bass_v3,
                 "4": _build_bass_v4, "5": _build_bass_v5}[ver]
        _CACHED["nc"] = build(emit)
        _CACHED["key"] = key
    nc = _CACHED["nc"]

    trace = os.environ.get("BASS_TRACE", "0") == "1"
    res = run_bass_kernel_spmd(
        nc, in_maps, core_ids=list(range(NCORES)), trace=trace
    )
    _CACHED["last_results"] = res
    _CACHED["nc_inmaps"] = (nc, in_maps)

    out = np.empty((Z, MUL, BASE), np.float32)
    for c in range(NCORES):
        o = _unpermute_core(res.results[c]["outp"])
        out[c * ZPC : (c + 1) * ZPC] = o[:ZPC]
    return out


# revision 15
# speedup vs baseline: 2.6448x; 1.2420x over previous
"""Trainium2 Bass kernel for nn_Contracter (per-channel bilinear CG contraction).

out[z,u,k] = sum_ij x1[z,u,i] * x2[z,u,j] * ww3j[u,k,i,j]
with ww3j[u,k,i,j] = sum_p weights[u,p] * w3j[p,k,i,j]

Z=50000 edges, u=64 channels, i/j/k=9 (0e+1o+2e irreps).

Strategy (8 cores, data-parallel over z):
 - Host folds weights into w3j (tiny) and permutes x1/x2 so that SBUF
   partitions hold (edge-half h, channel u) = 128 lanes, and the free dim
   holds (chunk, irrep-component i, edge-within-chunk). Per-channel CG
   coefficients then become per-partition scalars.
 - v1 (KVER=1): VectorE does everything: 71 tensor_tensor products + 83
   fused scalar_tensor_tensor MACs per chunk.
 - v2 (KVER=2): VectorE forms the 71 raw products; ScalarE applies
   the per-channel coefficient (83 scaled copies, float32r out); TensorE
   accumulates the scaled planes into per-k PSUM banks via an
   identity-stationary float32r matmul; ScalarE drains each bank.
 - v4 (KVER=4, default): per-channel coefficients ride the PE stationary as
   bf16 diagonal matrices; VectorE does only the 71 bf16 products (2x mode);
   TensorE does the 83 scaled accumulations; ScalarE drains PSUM.
 - Host inverse-permutes the output.
"""

import os
import numpy as np

import concourse.bacc as bacc
import concourse.mybir as mybir
from concourse.tile import TileContext
from concourse.bass_utils import run_bass_kernel_spmd

MUL = 64
BASE = 9
Z = 50000
NCORES = 8
ZPC = Z // NCORES          # 6250 edges per core
NCHUNK = 7                 # chunks per half
ZC = 448                   # edges per chunk per half
ZH = NCHUNK * ZC           # 3136 padded half
ZPAD = 2 * ZH              # 6272 padded edges per core

F32 = mybir.dt.float32
F32R = mybir.dt.float32r


def _emission_order(w3j):
    """Sparsity of the CG tensor: (i,j)-pair groups with their output k's."""
    nz = (np.abs(np.asarray(w3j)) > 0).any(axis=0)  # (k,i,j)
    by_pair = {}
    for k in range(BASE):
        for i in range(BASE):
            for j in range(BASE):
                if nz[k, i, j]:
                    by_pair.setdefault((i, j), []).append(k)
    return [(i, j, sorted(by_pair[(i, j)])) for (i, j) in sorted(by_pair)]


def _new_nc():
    return bacc.Bacc("TRN2", target_bir_lowering=False, debug=False)


def _build_bass(emit):
    ntrip = sum(len(ks) for (_, _, ks) in emit)
    nc = _new_nc()
    xind = nc.dram_tensor("xin", [128, NCHUNK, 2, BASE, ZC], F32, kind="ExternalInput")
    cfd = nc.dram_tensor("coef", [128, ntrip], F32, kind="ExternalInput")
    outd = nc.dram_tensor("outp", [128, NCHUNK, BASE, ZC], F32, kind="ExternalOutput")

    with TileContext(nc) as tc:
        with (
            tc.tile_pool(name="const", bufs=1) as cpool,
            tc.tile_pool(name="io", bufs=2) as iopool,
            tc.tile_pool(name="prod", bufs=4) as ppool,
        ):
            coeft = cpool.tile([128, ntrip], F32)
            nc.sync.dma_start(out=coeft[:], in_=cfd[:])

            for c in range(NCHUNK):
                xt = iopool.tile([128, 2, BASE, ZC], F32, tag="x")
                acct = iopool.tile([128, BASE, ZC], F32, tag="acc")
                nc.sync.dma_start(out=xt[:], in_=xind[:, c])
                x1t = xt[:, 0]
                x2t = xt[:, 1]
                nc.vector.memset(acct[:], 0.0)

                t = 0
                for (i, j, ks) in emit:
                    pt = ppool.tile([128, ZC], F32, tag="p")
                    nc.vector.tensor_mul(pt[:], x1t[:, i], x2t[:, j])
                    for k in ks:
                        sc = coeft[:, t : t + 1]
                        nc.vector.scalar_tensor_tensor(
                            out=acct[:, k],
                            in0=pt[:],
                            scalar=sc,
                            in1=acct[:, k],
                            op0=mybir.AluOpType.mult,
                            op1=mybir.AluOpType.add,
                        )
                        t += 1
                nc.sync.dma_start(out=outd[:, c], in_=acct[:])
    nc.compile()
    return nc


def _build_bass_v2(emit):
    """v2: VectorE forms raw products (71 TT); ScalarE applies the per-channel
    CG coefficient (83 scaled copies, fp32r out); TensorE accumulates the
    scaled planes into per-k PSUM banks via identity-stationary float32r
    matmuls; ScalarE drains each bank. k-major emission keeps PSUM usage low.
    """
    ntrip = sum(len(ks) for (_, _, ks) in emit)

    by_k = {k: [] for k in range(BASE)}
    for (i, j, ks) in emit:
        for k in ks:
            by_k[k].append((i, j))
    tidx = {}
    t = 0
    for k in range(BASE):
        for (i, j) in by_k[k]:
            tidx[(k, i, j)] = t
            t += 1

    nc = _new_nc()
    xind = nc.dram_tensor("xin", [128, NCHUNK, 2, BASE, ZC], F32, kind="ExternalInput")
    cfd = nc.dram_tensor("coef", [128, ntrip], F32, kind="ExternalInput")
    idd = nc.dram_tensor("ident", [128, 128], F32R, kind="ExternalInput")
    outd = nc.dram_tensor("outp", [128, NCHUNK, BASE, ZC], F32, kind="ExternalOutput")

    with TileContext(nc) as tc:
        with (
            tc.tile_pool(name="const", bufs=1) as cpool,
            tc.tile_pool(name="io", bufs=2) as iopool,
            tc.tile_pool(name="mprod", bufs=14) as mpool,
            tc.tile_pool(name="q", bufs=14) as qpool,
            tc.tile_pool(name="ps", bufs=3, space="PSUM") as pspool,
        ):
            coeft = cpool.tile([128, ntrip], F32)
            nc.sync.dma_start(out=coeft[:], in_=cfd[:])
            identt = cpool.tile([128, 128], F32R)
            nc.sync.dma_start(out=identt[:], in_=idd[:])

            for c in range(NCHUNK):
                xt = iopool.tile([128, 2, BASE, ZC], F32, tag="x")
                outt = iopool.tile([128, BASE, ZC], F32, tag="out")
                nc.sync.dma_start(out=xt[:], in_=xind[:, c])
                x1t = xt[:, 0]
                x2t = xt[:, 1]

                mprod = {}
                for k in range(BASE):
                    pairs_k = by_k[k]
                    ps = pspool.tile([128, ZC], F32, tag="acc")
                    for idx, (i, j) in enumerate(pairs_k):
                        sc = coeft[:, tidx[(k, i, j)] : tidx[(k, i, j)] + 1]
                        if (i, j) not in mprod:
                            mt = mpool.tile([128, ZC], F32, tag="m")
                            nc.vector.tensor_mul(mt[:], x1t[:, i], x2t[:, j])
                            mprod[(i, j)] = mt
                        qt = qpool.tile([128, ZC], F32R, tag="q")
                        nc.scalar.mul(qt[:], mprod[(i, j)][:], sc)
                        nc.tensor.matmul(
                            out=ps[:],
                            lhsT=identt[:],
                            rhs=qt[:],
                            start=(idx == 0),
                            stop=(idx == len(pairs_k) - 1),
                        )
                    nc.scalar.copy(outt[:, k], ps[:])
                nc.sync.dma_start(out=outd[:, c], in_=outt[:])
    nc.compile()
    return nc




BF16 = mybir.dt.bfloat16


def _build_bass_v3(emit):
    """v3: like v2 but bf16 inputs/products for DVE 2x mode. Single-k pairs
    fuse product+coefficient into one scalar_tensor_tensor on VectorE;
    multi-k pairs take a raw TT product + per-k ScalarE scaled copies.
    TensorE accumulates bf16 planes into fp32 PSUM via identity matmuls."""
    ntrip = sum(len(ks) for (_, _, ks) in emit)

    pair_ks = {(i, j): ks for (i, j, ks) in emit}
    by_k = {k: [] for k in range(BASE)}
    for (i, j, ks) in emit:
        for k in ks:
            by_k[k].append((i, j))
    tidx = {}
    t = 0
    for k in range(BASE):
        for (i, j) in by_k[k]:
            tidx[(k, i, j)] = t
            t += 1

    nc = _new_nc()
    xind = nc.dram_tensor("xin", [128, NCHUNK, 2, BASE, ZC], BF16, kind="ExternalInput")
    cfd = nc.dram_tensor("coef", [128, ntrip], F32, kind="ExternalInput")
    idd = nc.dram_tensor("ident", [128, 128], BF16, kind="ExternalInput")
    outd = nc.dram_tensor("outp", [128, NCHUNK, BASE, ZC], F32, kind="ExternalOutput")

    with TileContext(nc) as tc:
        with (
            tc.tile_pool(name="const", bufs=1) as cpool,
            tc.tile_pool(name="io", bufs=2) as iopool,
            tc.tile_pool(name="mprod", bufs=14) as mpool,
            tc.tile_pool(name="q", bufs=14) as qpool,
            tc.tile_pool(name="ps", bufs=3, space="PSUM") as pspool,
        ):
            coeft = cpool.tile([128, ntrip], F32)
            nc.sync.dma_start(out=coeft[:], in_=cfd[:])
            identt = cpool.tile([128, 128], BF16)
            nc.sync.dma_start(out=identt[:], in_=idd[:])

            for c in range(NCHUNK):
                xt = iopool.tile([128, 2, BASE, ZC], BF16, tag="x")
                outt = iopool.tile([128, BASE, ZC], F32, tag="out")
                nc.sync.dma_start(out=xt[:], in_=xind[:, c])
                x1t = xt[:, 0]
                x2t = xt[:, 1]

                mprod = {}
                for k in range(BASE):
                    pairs_k = by_k[k]
                    ps = pspool.tile([128, ZC], F32, tag="acc")
                    for idx, (i, j) in enumerate(pairs_k):
                        sc = coeft[:, tidx[(k, i, j)] : tidx[(k, i, j)] + 1]
                        if (i, j) not in mprod:
                            mt = mpool.tile([128, ZC], BF16, tag="m")
                            nc.vector.tensor_mul(mt[:], x1t[:, i], x2t[:, j])
                            mprod[(i, j)] = mt
                        qt = qpool.tile([128, ZC], BF16, tag="q")
                        nc.scalar.mul(qt[:], mprod[(i, j)][:], sc)
                        nc.tensor.matmul(
                            out=ps[:],
                            lhsT=identt[:],
                            rhs=qt[:],
                            start=(idx == 0),
                            stop=(idx == len(pairs_k) - 1),
                        )
                    nc.scalar.copy(outt[:, k], ps[:])
                nc.sync.dma_start(out=outd[:, c], in_=outt[:])
    nc.compile()
    return nc




def _build_bass_v4(emit):
    """v4: per-channel coefficients ride the PE stationary as diagonal
    matrices (bf16). VectorE: 71 raw bf16 products. TensorE: 83
    diag-stationary matmuls accumulating into per-k PSUM banks. ScalarE:
    9 PSUM drains. No per-triple scaling op on any engine."""
    ntrip = sum(len(ks) for (_, _, ks) in emit)

    by_k = {k: [] for k in range(BASE)}
    for (i, j, ks) in emit:
        for k in ks:
            by_k[k].append((i, j))
    tidx = {}
    t = 0
    for k in range(BASE):
        for (i, j) in by_k[k]:
            tidx[(k, i, j)] = t
            t += 1

    nc = _new_nc()
    xind = nc.dram_tensor("xin", [128, NCHUNK, 2, BASE, ZC], BF16, kind="ExternalInput")
    dgd = nc.dram_tensor("diags", [128, ntrip, 128], BF16, kind="ExternalInput")
    outd = nc.dram_tensor("outp", [128, NCHUNK, BASE, ZC], F32, kind="ExternalOutput")

    with TileContext(nc) as tc:
        with (
            tc.tile_pool(name="const", bufs=1) as cpool,
            tc.tile_pool(name="io", bufs=2) as iopool,
            tc.tile_pool(name="mprod", bufs=14) as mpool,
            tc.tile_pool(name="ps", bufs=3, space="PSUM") as pspool,
        ):
            diagt = cpool.tile([128, ntrip, 128], BF16)
            nc.sync.dma_start(out=diagt[:], in_=dgd[:])

            for c in range(NCHUNK):
                xt = iopool.tile([128, 2, BASE, ZC], BF16, tag="x")
                outt = iopool.tile([128, BASE, ZC], F32, tag="out")
                nc.sync.dma_start(out=xt[:], in_=xind[:, c])
                x1t = xt[:, 0]
                x2t = xt[:, 1]

                mprod = {}
                for k in range(BASE):
                    pairs_k = by_k[k]
                    ps = pspool.tile([128, ZC], F32, tag="acc")
                    for idx, (i, j) in enumerate(pairs_k):
                        if (i, j) not in mprod:
                            mt = mpool.tile([128, ZC], BF16, tag="m")
                            nc.vector.tensor_mul(mt[:], x1t[:, i], x2t[:, j])
                            mprod[(i, j)] = mt
                        nc.tensor.matmul(
                            out=ps[:],
                            lhsT=diagt[:, tidx[(k, i, j)]],
                            rhs=mprod[(i, j)][:],
                            start=(idx == 0),
                            stop=(idx == len(pairs_k) - 1),
                        )
                    nc.scalar.copy(outt[:, k], ps[:])
                nc.sync.dma_start(out=outd[:, c], in_=outt[:])
    nc.compile()
    return nc




def _build_bass_v5(emit):
    """v5: precision-safe diag variant. VectorE: 71 fp32 products written as
    float32r; TensorE: 83 float32r diag-stationary matmuls (1 cyc/col at
    N>=256); ScalarE: 9 PSUM drains. Error stays at fp32r rounding level."""
    ntrip = sum(len(ks) for (_, _, ks) in emit)

    by_k = {k: [] for k in range(BASE)}
    for (i, j, ks) in emit:
        for k in ks:
            by_k[k].append((i, j))
    tidx = {}
    t = 0
    for k in range(BASE):
        for (i, j) in by_k[k]:
            tidx[(k, i, j)] = t
            t += 1

    nc = _new_nc()
    xind = nc.dram_tensor("xin", [128, NCHUNK, 2, BASE, ZC], F32, kind="ExternalInput")
    dgd = nc.dram_tensor("diags", [128, ntrip, 128], F32R, kind="ExternalInput")
    outd = nc.dram_tensor("outp", [128, NCHUNK, BASE, ZC], F32, kind="ExternalOutput")

    with TileContext(nc) as tc:
        with (
            tc.tile_pool(name="const", bufs=1) as cpool,
            tc.tile_pool(name="io", bufs=2) as iopool,
            tc.tile_pool(name="mprod", bufs=14) as mpool,
            tc.tile_pool(name="ps", bufs=3, space="PSUM") as pspool,
        ):
            diagt = cpool.tile([128, ntrip, 128], F32R)
            nc.sync.dma_start(out=diagt[:], in_=dgd[:])

            for c in range(NCHUNK):
                xt = iopool.tile([128, 2, BASE, ZC], F32, tag="x")
                outt = iopool.tile([128, BASE, ZC], F32, tag="out")
                nc.sync.dma_start(out=xt[:], in_=xind[:, c])
                x1t = xt[:, 0]
                x2t = xt[:, 1]

                mprod = {}
                for k in range(BASE):
                    pairs_k = by_k[k]
                    ps = pspool.tile([128, ZC], F32, tag="acc")
                    for idx, (i, j) in enumerate(pairs_k):
                        if (i, j) not in mprod:
                            mt = mpool.tile([128, ZC], F32R, tag="m")
                            nc.vector.tensor_mul(mt[:], x1t[:, i], x2t[:, j])
                            mprod[(i, j)] = mt
                        nc.tensor.matmul(
                            out=ps[:],
                            lhsT=diagt[:, tidx[(k, i, j)]],
                            rhs=mprod[(i, j)][:],
                            start=(idx == 0),
                            stop=(idx == len(pairs_k) - 1),
                        )
                    nc.scalar.copy(outt[:, k], ps[:])
                nc.sync.dma_start(out=outd[:, c], in_=outt[:])
    nc.compile()
    return nc


def _coef_order_v2(emit):
    by_k = {k: [] for k in range(BASE)}
    for (i, j, ks) in emit:
        for k in ks:
            by_k[k].append((i, j))
    return [(k, i, j) for k in range(BASE) for (i, j) in by_k[k]]


def _coef_order_v1(emit):
    return [(k, i, j) for (i, j, ks) in emit for k in ks]


_CACHED = {}


def _permute_core(x_core_pad):
    """(ZPAD, 64, 9) -> (128, NCHUNK, 9, ZC) with partition p = h*64+u."""
    v = x_core_pad.reshape(2, NCHUNK, ZC, MUL, BASE)
    v = v.transpose(0, 3, 1, 4, 2)  # (h, u, chunk, i, zl)
    return np.ascontiguousarray(v.reshape(128, NCHUNK, BASE, ZC), dtype=np.float32)


def _unpermute_core(o_dev):
    """(128, NCHUNK, 9, ZC) -> (ZPAD, 64, 9)."""
    v = o_dev.reshape(2, MUL, NCHUNK, BASE, ZC)
    v = v.transpose(0, 2, 4, 1, 3)  # (h, chunk, zl, u, k)
    return v.reshape(ZPAD, MUL, BASE)


def kernel(x1, x2, weights, w3j):
    x1 = np.asarray(x1, dtype=np.float32)
    x2 = np.asarray(x2, dtype=np.float32)
    weights = np.asarray(weights, dtype=np.float32)
    w3j = np.asarray(w3j, dtype=np.float32)

    ver = os.environ.get("KVER", "4")

    # fold path weights into the CG tensor (tiny host einsum)
    ww3j = np.einsum("up,pkij->ukij", weights, w3j).astype(np.float32)

    emit = _emission_order(w3j)
    order = _coef_order_v1(emit) if ver == "1" else _coef_order_v2(emit)
    coef_u = np.stack([ww3j[:, k, i, j] for (k, i, j) in order], axis=1)  # (64,T)
    coef = np.ascontiguousarray(
        np.concatenate([coef_u, coef_u], axis=0), dtype=np.float32
    )  # (128, T)
    import ml_dtypes
    idt = np.float32 if ver == "2" else ml_dtypes.bfloat16
    ident = np.ascontiguousarray(np.eye(128, dtype=idt))
    if ver in ("4", "5"):
        T = coef.shape[1]
        diags = np.zeros((128, T, 128), np.float32)
        diags[np.arange(128)[:, None], np.arange(T)[None, :], np.arange(128)[:, None]] = coef
        if ver == "4":
            diags = np.ascontiguousarray(diags.astype(ml_dtypes.bfloat16))
        else:
            diags = np.ascontiguousarray(diags)

    x1r = x1.reshape(Z, MUL, BASE)
    x2r = x2.reshape(Z, MUL, BASE)

    in_maps = []
    for c in range(NCORES):
        sl = slice(c * ZPC, (c + 1) * ZPC)
        x1c = np.zeros((ZPAD, MUL, BASE), np.float32)
        x2c = np.zeros((ZPAD, MUL, BASE), np.float32)
        x1c[:ZPC] = x1r[sl]
        x2c[:ZPC] = x2r[sl]
        xin = np.ascontiguousarray(
            np.stack([_permute_core(x1c), _permute_core(x2c)], axis=2)
        )  # (128, NCHUNK, 2, BASE, ZC)
        if ver in ("3", "4"):
            xin = xin.astype(ml_dtypes.bfloat16)
        if ver == "1":
            in_maps.append({"xin": xin, "coef": coef})
        elif ver in ("4", "5"):
            in_maps.append({"xin": xin, "diags": diags})
        else:
            in_maps.append({"xin": xin, "coef": coef, "ident": ident})

    key = (ver,) + tuple((i, j, tuple(ks)) for (i, j, ks) in emit)
    if _CACHED.get("key") != key:
        build = {"1": _build_bass, "2": _build_bass_v2, "3": _build_bass_v3,
                 "4": _build_bass_v4, "5": _build_bass_v5}[ver]
        _CACHED["nc"] = build(emit)
        _CACHED["key"] = key
    nc = _CACHED["nc"]

    trace = os.environ.get("BASS_TRACE", "0") == "1"
    res = run_bass_kernel_spmd(
        nc, in_maps, core_ids=list(range(NCORES)), trace=trace
    )
    _CACHED["last_results"] = res
    _CACHED["nc_inmaps"] = (nc, in_maps)

    out = np.empty((Z, MUL, BASE), np.float32)
    for c in range(NCORES):
        o = _unpermute_core(res.results[c]["outp"])
        out[c * ZPC : (c + 1) * ZPC] = o[:ZPC]
    return out
